# revision 18
# baseline (speedup 1.0000x reference)
"""RBF kernel matrix K[i,j] = exp(-||x_i - y_j||^2) on 8 trn2 NeuronCores.

Strategy (hardcoded for x:[8192,256] f32, y:[8192,256] f32):
  - Shard rows of x across the 8 cores (1024 rows each); replicate y.
  - -dist2 = (x . 2y) - x2_i - y2_j as one augmented GEMM on the tensor
    engine: two fp8 K=128 passes for (x . 2y) plus a bf16 K=2 pass with
    xe=[1;x2], ye=[-y2;-1] for the rank-2 norm term; then exp() on the
    scalar engine straight out of PSUM.
  - Host precomputes fp8/bf16 operands PRE-TILED to the SBUF layout
    (partition-major, per-partition contiguous, y in g-major chunk
    order) so input DMAs use large descriptors, and xe is packed into
    the ye tensor so the whole input side is 5 DMAs over 3 rings
    (sync / scalar / gpsimd) - per-ring completion fences (~2us each)
    overlap instead of stacking up.

Perf structure (the kernel is bound by the 33.55 MB/core output write):
  - t~0: gpsimd memsets feed dependency-free warmup matmuls that bridge
    the PE HAM clock gate until the first inputs land, and a dummy exp
    pulls the ACT table load forward.
  - Block 0's first group is computed and stored per 1024 cols so the
    output stream starts right behind the input stream.
  - Output stores are 1 MB, alternating between the sync and scalar
    HWDGE rings; the final group is split so all DMA engines finish
    together.
"""

import numpy as np
import ml_dtypes

N = 8192
M = 8192
D = 256
NCORES = 8
RPC = N // NCORES  # rows of x per core: 1024

_cached = {}


def _build():
    import concourse.tile as tile
    import concourse.mybir as mybir
    from concourse import bacc

    f32 = mybir.dt.float32
    bf16 = mybir.dt.bfloat16
    fp8 = mybir.dt.float8e4

    nc = bacc.Bacc("TRN2", target_bir_lowering=False)

    JT = 512          # matmul free dim (one PSUM bank)
    JG = 2048         # PSUM group: 4 banks
    NIG = RPC // 128  # 8 i-blocks
    NG = M // JG      # 4 j-groups per block
    KC = D // 128     # 2 contraction chunks
    NWARM = 14        # HAM warmup matmuls (~6us cold bridge)

    # Pre-tiled inputs (partition-major, per-partition contiguous):
    #   xTt[p, c*RPC + i]         = x[i, c*128 + p]
    #   yTt[p, ((g*KC)+c)*JG + j] = 2*y[g*JG + j, c*128 + p]
    #   yep = [ye | xe]: ye[k, j] over cols 0:M, xe[k, i] at M + i
    #     with xe=[1; x2], ye=[-y2; -1]
    xTt = nc.dram_tensor("xTt", [128, KC * RPC], fp8, kind="ExternalInput")
    yTt = nc.dram_tensor("yTt", [128, NG * KC * JG], fp8, kind="ExternalInput")
    yep = nc.dram_tensor("yep", [2, M + RPC], bf16, kind="ExternalInput")
    out = nc.dram_tensor("out", [RPC, M], f32, kind="ExternalOutput")

    xT_ap = xTt[:].rearrange("p (c f) -> p c f", c=KC)
    yT_ap = yTt[:].rearrange("p (g c f) -> p g c f", g=NG, c=KC)
    out_ap = out[:].rearrange("(g p) f -> g p f", p=128)

    with tile.TileContext(nc) as tc:
        with (
            tc.tile_pool(name="consts", bufs=1) as consts,
            tc.tile_pool(name="outsb", bufs=6) as outsb,
            tc.tile_pool(name="psum", bufs=2, space="PSUM") as psum,
        ):
            # ---- dependency-free warmup, scheduled at t~0 ----
            wsrc = consts.tile([128, JT], fp8)
            nc.gpsimd.memset(wsrc[:], 0)
            wein = consts.tile([128, 16], f32)
            nc.gpsimd.memset(wein[:], -300.0)
            weout = consts.tile([128, 16], f32)

            # ---- inputs: 5 DMAs over 3 rings, first-needed first ----
            yT_sb = consts.tile([128, NG, KC, JG], fp8)
            nc.scalar.dma_start(yT_sb[:, 0], yT_ap[:, 0])
            xT_sb = consts.tile([128, KC, RPC], fp8)
            nc.sync.dma_start(xT_sb[:], xT_ap)
            yep_sb = consts.tile([2, M + RPC], bf16)
            nc.gpsimd.dma_start(yep_sb[:], yep[:])
            nc.scalar.dma_start(yT_sb[:, 1], yT_ap[:, 1])
            nc.sync.dma_start(
                yT_sb[:, 2:4].rearrange("p g c f -> p (g c f)"),
                yT_ap[:, 2:4].rearrange("p g c f -> p (g c f)"),
            )

            # exp table load + PE clock-gate warmup while inputs stream
            nc.scalar.activation(weout[:], wein[:],
                                 mybir.ActivationFunctionType.Exp)
            wp = psum.tile([128, JG], f32, tag="pt")
            for _ in range(NWARM):
                nc.tensor.matmul(wp[:, :JT], lhsT=wsrc[:, :128], rhs=wsrc[:],
                                 start=True, stop=True)

            stores = 0

            def store(ig, j0, j1, ot, oj0):
                nonlocal stores
                eng = nc.sync if stores % 2 == 0 else nc.scalar
                stores += 1
                eng.dma_start(out_ap[ig, :, j0:j1], ot[:, j0 - oj0:j1 - oj0])

            def mm_tile(ig, g, pt, jj):
                i0 = ig * 128
                sl = slice(jj * JT, (jj + 1) * JT)
                for c in range(KC):
                    nc.tensor.matmul(
                        pt[:, sl],
                        lhsT=xT_sb[:, c, i0:i0 + 128],
                        rhs=yT_sb[:, g, c, jj * JT:(jj + 1) * JT],
                        start=(c == 0), stop=False,
                    )
                nc.tensor.matmul(
                    pt[:, sl],
                    lhsT=yep_sb[:, M + i0:M + i0 + 128],
                    rhs=yep_sb[:, g * JG + jj * JT:g * JG + (jj + 1) * JT],
                    start=False, stop=True,
                )

            for ig in range(NIG):
                i0 = ig * 128
                for g in range(NG):
                    j0 = g * JG
                    ot = outsb.tile([128, JG], f32)
                    pt = psum.tile([128, JG], f32, tag="pt")
                    if ig == 0 and g < 2:
                        # tile-major halves: exp + store per 1024 cols so
                        # the output stream starts right behind the input
                        # stream
                        for h in range(2):
                            for jj in (2 * h, 2 * h + 1):
                                mm_tile(ig, g, pt, jj)
                            hs = slice(j0 + h * 1024, j0 + (h + 1) * 1024)
                            nc.scalar.activation(
                                ot[:, h * 1024:(h + 1) * 1024],
                                pt[:, h * 1024:(h + 1) * 1024],
                                mybir.ActivationFunctionType.Exp,
                            )
                            store(ig, hs.start, hs.stop, ot, j0)
                    else:
                        # c-major: weight reloads amortized over 4 tiles
                        for c in range(KC):
                            for jj in range(JG // JT):
                                sl = slice(jj * JT, (jj + 1) * JT)
                                nc.tensor.matmul(
                                    pt[:, sl],
                                    lhsT=xT_sb[:, c, i0:i0 + 128],
                                    rhs=yT_sb[:, g, c, jj * JT:(jj + 1) * JT],
                                    start=(c == 0), stop=False,
                                )
                        for jj in range(JG // JT):
                            sl = slice(jj * JT, (jj + 1) * JT)
                            nc.tensor.matmul(
                                pt[:, sl],
                                lhsT=yep_sb[:, M + i0:M + i0 + 128],
                                rhs=yep_sb[:, j0 + jj * JT:j0 + (jj + 1) * JT],
                                start=False, stop=True,
                            )
                        nc.scalar.activation(
                            ot[:], pt[:],
                            mybir.ActivationFunctionType.Exp,
                        )
                        if ig == NIG - 1 and g == NG - 1:
                            # split the final store so both rings and all
                            # DMA engines finish together (no skew tail)
                            store(ig, j0, j0 + 1024, ot, j0)
                            store(ig, j0 + 1024, j0 + JG, ot, j0)
                        else:
                            store(ig, j0, j0 + JG, ot, j0)

    nc.compile()
    return nc


def _prep_inputs(x: np.ndarray, y: np.ndarray):
    bf16 = ml_dtypes.bfloat16
    fp8 = ml_dtypes.float8_e4m3
    x = np.asarray(x, dtype=np.float32)
    y = np.asarray(y, dtype=np.float32)
    x2 = np.sum(x * x, axis=1)  # [N]
    y2 = np.sum(y * y, axis=1)  # [M]
    NG, KC, JG, NIG = 4, 2, 2048, 8

    # yTt[p, g, c, j] = 2*y[g*JG+j, c*128+p]
    yT = np.transpose(2.0 * y).astype(fp8)            # [D, M]
    yTt = np.ascontiguousarray(
        yT.reshape(KC, 128, NG, JG).transpose(1, 2, 0, 3).reshape(128, -1)
    )

    in_maps = []
    for c in range(NCORES):
        sl = slice(c * RPC, (c + 1) * RPC)
        xT_c = np.transpose(x[sl]).astype(fp8)        # [D, RPC]
        xTt_c = np.ascontiguousarray(
            xT_c.reshape(2, 128, RPC).transpose(1, 0, 2).reshape(128, 2 * RPC)
        )
        yep_c = np.empty((2, M + RPC), dtype=bf16)
        yep_c[0, :M] = (-y2).astype(bf16)   # ye row 0: -y2_j
        yep_c[1, :M] = bf16(-1.0)           # ye row 1: -1
        yep_c[0, M:] = bf16(1.0)            # xe row 0: 1
        yep_c[1, M:] = x2[sl].astype(bf16)  # xe row 1: x2_i
        in_maps.append({"xTt": xTt_c, "yTt": yTt, "yep": yep_c})
    return in_maps


def kernel(x: np.ndarray, y: np.ndarray, _trace: bool = False):
    from concourse.bass_utils import run_bass_kernel_spmd

    if "nc" not in _cached:
        _cached["nc"] = _build()
    nc = _cached["nc"]

    in_maps = _prep_inputs(x, y)
    res = run_bass_kernel_spmd(
        nc, in_maps, core_ids=list(range(NCORES)), trace=_trace
    )
    outp = np.concatenate([res.results[c]["out"] for c in range(NCORES)], axis=0)
    if _trace:
        _cached["last_result"] = res
    return outp


# revision 19
# speedup vs baseline: 1.0815x; 1.0815x over previous
"""RBF kernel matrix K[i,j] = exp(-||x_i - y_j||^2) on 8 trn2 NeuronCores.

Strategy (hardcoded for x:[8192,256] f32, y:[8192,256] f32):
  - Shard rows of x across the 8 cores (1024 rows each); replicate y.
  - Use the expansion -dist2 = (x . 2y) - x2_i - y2_j, computed as one
    augmented GEMM on the tensor engine:
        PSUM[i,j] = sum_d xT[d,i] * yT2[d,j]  +  xe[:,i] . ye[:,j]
    where xe = [x2_i; 1], ye = [-1; -y2_j] (a K=2 matmul adds the rank-2
    bias term), then exp() on the scalar engine straight out of PSUM.
  - Host precomputes fp8/bf16 operands PRE-TILED to the SBUF layout
    ([128 partitions, ...] with per-partition contiguous bytes) so the
    input DMAs use large descriptors and stream at full HBM rate.

Perf structure (the kernel is bound by the 33.55 MB/core output write):
  - t~0: gpsimd memsets feed dependency-free warmup matmuls (flip the
    PE HAM clock gate) and a dummy exp (pull the ACT table load) while
    the inputs stream.
  - Inputs split across the two HWDGE rings; y chunked so the first
    matmul group's operands land first.
  - Block 0 is stored in 1024-col slices to start the output stream
    right behind the input stream; blocks 1-7 are stored as single
    contiguous 4 MB stores, alternating between the sync and scalar
    rings so issue costs and completion fences overlap.
"""

import numpy as np
import ml_dtypes

N = 8192
M = 8192
D = 256
NCORES = 8
RPC = N // NCORES  # rows of x per core: 1024

_cached = {}


def _build():
    import concourse.tile as tile
    import concourse.mybir as mybir
    from concourse import bacc

    f32 = mybir.dt.float32
    bf16 = mybir.dt.bfloat16
    fp8 = mybir.dt.float8e4

    nc = bacc.Bacc("TRN2", target_bir_lowering=False)

    JT = 512          # matmul free dim (one PSUM bank)
    JG = 2048         # PSUM group: 4 banks
    NIG = RPC // 128  # 8 i-blocks
    NG = M // JG      # 4 j-groups per block
    KC = D // 128     # 2 contraction chunks
    NWARM = 8         # HAM warmup matmuls (~3.4us cold)

    # Pre-tiled inputs: partition-major, per-partition contiguous.
    #   xTt[p, c*RPC + i] = x[i, c*128 + p]
    #   yTt[p, c*M + j]   = 2*y[j, c*128 + p]
    xTt = nc.dram_tensor("xTt", [128, KC * RPC], fp8, kind="ExternalInput")
    xe = nc.dram_tensor("xe", [2, RPC], bf16, kind="ExternalInput")
    yTt = nc.dram_tensor("yTt", [128, KC * M], fp8, kind="ExternalInput")
    ye = nc.dram_tensor("ye", [2, M], bf16, kind="ExternalInput")
    out = nc.dram_tensor("out", [RPC, M], f32, kind="ExternalOutput")

    xT_ap = xTt[:].rearrange("p (c f) -> p c f", c=KC)
    yT_ap = yTt[:].rearrange("p (c f) -> p c f", c=KC)
    out_ap = out[:].rearrange("(g p) f -> g p f", p=128)

    JSPL = 2048  # first-j split point for the chunked y load

    with tile.TileContext(nc) as tc:
        with (
            tc.tile_pool(name="consts", bufs=1) as consts,
            tc.tile_pool(name="outsb", bufs=6) as outsb,
            tc.tile_pool(name="psum", bufs=2, space="PSUM") as psum,
        ):
            # ---- dependency-free warmup, scheduled at t~0 ----
            wsrc = consts.tile([128, JT], fp8)
            nc.gpsimd.memset(wsrc[:], 0)
            wein = consts.tile([128, 16], f32)
            nc.gpsimd.memset(wein[:], -300.0)
            weout = consts.tile([128, 16], f32)

            # ---- inputs: x-side on sync ring, y-side on scalar ring ----
            xT_sb = consts.tile([128, KC, RPC], fp8)
            nc.sync.dma_start(xT_sb[:], xT_ap)
            xe_sb = consts.tile([2, RPC], bf16)
            nc.sync.dma_start(xe_sb[:], xe[:])
            ye_sb = consts.tile([2, M], bf16)
            nc.scalar.dma_start(ye_sb[:], ye[:])
            yT_sb = consts.tile([128, KC, M], fp8)
            nc.scalar.dma_start(yT_sb[:, 0, 0:JSPL], yT_ap[:, 0, 0:JSPL])
            nc.scalar.dma_start(yT_sb[:, 1, 0:JSPL], yT_ap[:, 1, 0:JSPL])
            nc.scalar.dma_start(yT_sb[:, 0, JSPL:M], yT_ap[:, 0, JSPL:M])
            nc.scalar.dma_start(yT_sb[:, 1, JSPL:M], yT_ap[:, 1, JSPL:M])

            # exp table load + PE clock-gate warmup while inputs stream
            nc.scalar.activation(weout[:], wein[:],
                                 mybir.ActivationFunctionType.Exp)
            wp = psum.tile([128, JG], f32, tag="pt")
            for _ in range(NWARM):
                nc.tensor.matmul(wp[:, :JT], lhsT=wsrc[:, :128], rhs=wsrc[:],
                                 start=True, stop=True)

            stores = 0

            def store(ig, j0, j1, ot, oj0):
                nonlocal stores
                eng = nc.sync if stores % 2 == 0 else nc.scalar
                stores += 1
                eng.dma_start(out_ap[ig, :, j0:j1], ot[:, j0 - oj0:j1 - oj0])

            for ig in range(NIG):
                i0 = ig * 128
                for g in range(NG):
                    j0 = g * JG
                    ot = outsb.tile([128, JG], f32)
                    pt = psum.tile([128, JG], f32, tag="pt")
                    # c-major: weight reloads amortized over 4 tiles
                    for c in range(KC):
                        for jj in range(JG // JT):
                            sl = slice(jj * JT, (jj + 1) * JT)
                            nc.tensor.matmul(
                                pt[:, sl],
                                lhsT=xT_sb[:, c, i0:i0 + 128],
                                rhs=yT_sb[:, c, j0 + jj * JT:j0 + (jj + 1) * JT],
                                start=(c == 0), stop=False,
                            )
                    for jj in range(JG // JT):
                        sl = slice(jj * JT, (jj + 1) * JT)
                        nc.tensor.matmul(
                            pt[:, sl],
                            lhsT=xe_sb[:, i0:i0 + 128],
                            rhs=ye_sb[:, j0 + jj * JT:j0 + (jj + 1) * JT],
                            start=False, stop=True,
                        )
                    if ig == 0 and g < 2:
                        # exp + store per 1024 cols: output stream starts
                        # right behind the input stream
                        for h in range(2):
                            hs = slice(j0 + h * 1024, j0 + (h + 1) * 1024)
                            nc.scalar.activation(
                                ot[:, h * 1024:(h + 1) * 1024],
                                pt[:, h * 1024:(h + 1) * 1024],
                                mybir.ActivationFunctionType.Exp,
                            )
                            store(ig, hs.start, hs.stop, ot, j0)
                    else:
                        nc.scalar.activation(
                            ot[:], pt[:],
                            mybir.ActivationFunctionType.Exp,
                        )
                        store(ig, j0, j0 + JG, ot, j0)

    nc.compile()
    return nc


def _prep_inputs(x: np.ndarray, y: np.ndarray):
    bf16 = ml_dtypes.bfloat16
    fp8 = ml_dtypes.float8_e4m3
    x = np.asarray(x, dtype=np.float32)
    y = np.asarray(y, dtype=np.float32)
    x2 = np.sum(x * x, axis=1)  # [N]
    y2 = np.sum(y * y, axis=1)  # [M]

    # yTt[p, c*M + j] = 2*y[j, c*128 + p]  (partition-major, contiguous j)
    yT = np.transpose(2.0 * y).astype(fp8)            # [D, M]
    yTt = np.ascontiguousarray(
        yT.reshape(2, 128, M).transpose(1, 0, 2).reshape(128, 2 * M)
    )
    ye = np.empty((2, M), dtype=bf16)
    ye[0] = bf16(-1.0)
    ye[1] = (-y2).astype(bf16)

    in_maps = []
    for c in range(NCORES):
        sl = slice(c * RPC, (c + 1) * RPC)
        xT_c = np.transpose(x[sl]).astype(fp8)        # [D, RPC]
        xTt_c = np.ascontiguousarray(
            xT_c.reshape(2, 128, RPC).transpose(1, 0, 2).reshape(128, 2 * RPC)
        )
        xe_c = np.empty((2, RPC), dtype=bf16)
        xe_c[0] = x2[sl].astype(bf16)
        xe_c[1] = bf16(1.0)
        in_maps.append({"xTt": xTt_c, "xe": xe_c, "yTt": yTt, "ye": ye})
    return in_maps


def kernel(x: np.ndarray, y: np.ndarray, _trace: bool = False):
    from concourse.bass_utils import run_bass_kernel_spmd

    if "nc" not in _cached:
        _cached["nc"] = _build()
    nc = _cached["nc"]

    in_maps = _prep_inputs(x, y)
    res = run_bass_kernel_spmd(
        nc, in_maps, core_ids=list(range(NCORES)), trace=_trace
    )
    outp = np.concatenate([res.results[c]["out"] for c in range(NCORES)], axis=0)
    if _trace:
        _cached["last_result"] = res
    return outp
